# revision 14
# baseline (speedup 1.0000x reference)
"""Conv4d (B=2, Ci=32, Co=64, 16^4 spatial, k=3^4, stride 1, pad 1) on 8
Trainium2 NeuronCores.

Sharding: 8 cores = batch(2) x T-quarters(4). Each core computes
out[64co, 4t, 16d, 16h, 16w] for its (b, t-quarter).

The 81 taps are covered by three passes sized to keep the tensor engine
instruction count low (the ~31ns/inst issue rate binds at K=32):
  A: (kt,kd) in {(0,0),(0,1),(0,2),(1,0)} packed into K=128 (partition
     group g holds x shifted by combo g), M=64, one matmul per (kh,kw).
  B: (kt,kd) in {(1,2),(2,0),(2,1),(2,2)} likewise on a second layout.
  C: (kt,kd)=(1,1) as K=32 matmuls on a cropped quadrant layout
     (partition group r = D-quarter), 4 row groups concurrent, issued
     in two unit-waves so epilogues stagger.
Each pass splits its 9 (kh,kw) taps across the two 64-wide PE column
groups (tile_position col 0 / 64), accumulating even taps into PSUM
partitions 0-63 and odd taps into 64-127 of one [128,512] bank per
(to, dp) output unit; 8 units (one `to` batch) live at once = 8 banks.
Epilogue: ACT adds bias to the odd half, DVE adds the halves, DMA out.
"""
import sys

sys.path.insert(0, "/opt/trn_rl_repo")
import numpy as np
import ml_dtypes

N_CORES = 8
KHW = [(kh, kw) for kh in range(3) for kw in range(3)]
A_COMBOS = [(0, 0), (0, 1), (0, 2), (1, 0)]
B_COMBOS = [(1, 2), (2, 0), (2, 1), (2, 2)]

_NC = None


def _build():
    global _NC
    if _NC is not None:
        return _NC
    import concourse.bacc as bacc
    import concourse.tile as tile
    from concourse import mybir

    f32 = mybir.dt.float32
    bf16 = mybir.dt.bfloat16

    nc = bacc.Bacc("TRN2", debug=False, target_bir_lowering=False,
                   num_devices=N_CORES)
    xqa = nc.dram_tensor("xqa", [128, 20736], bf16, kind="ExternalInput")
    xqb = nc.dram_tensor("xqb", [128, 20736], bf16, kind="ExternalInput")
    xqc = nc.dram_tensor("xqc", [128, 5184], bf16, kind="ExternalInput")
    wa = nc.dram_tensor("wa", [128, 576], bf16, kind="ExternalInput")
    wb = nc.dram_tensor("wb", [128, 576], bf16, kind="ExternalInput")
    wc = nc.dram_tensor("wc", [128, 576], bf16, kind="ExternalInput")
    bq = nc.dram_tensor("biasq", [128, 1], f32, kind="ExternalInput")
    out = nc.dram_tensor("out", [64, 16384], f32, kind="ExternalOutput")

    with tile.TileContext(nc) as tc:
        with tc.tile_pool(name="xp", bufs=1) as xp, \
             tc.tile_pool(name="wp", bufs=1) as wp, \
             tc.tile_pool(name="op", bufs=8) as op_, \
             tc.tile_pool(name="pp", bufs=8, space="PSUM") as pp:
            wat = wp.tile([128, 576], bf16)
            wbt = wp.tile([128, 576], bf16)
            wct = wp.tile([128, 576], bf16)
            btile = wp.tile([128, 1], f32)
            xat = xp.tile([128, 20736], bf16)
            xbt = xp.tile([128, 20736], bf16)
            xct = xp.tile([128, 5184], bf16)

            # Issue order == arrival order (one FIFO input queue feeding
            # all 16 DMA engines). First matmuls need wa + A[t0=0,d 0..3]
            # only, so those go first, d-chunked, on the sync queue in
            # case it starts ahead of gpsimd.
            nc.sync.dma_start(wat[:], wa.ap()[:])
            for q in range(4):
                nc.sync.dma_start(xat[:, q * 1296:(q + 1) * 1296],
                                  xqa.ap()[:, q * 1296:(q + 1) * 1296])
            nc.gpsimd.dma_start(wbt[:], wb.ap()[:])
            for q in range(2):
                nc.gpsimd.dma_start(
                    xbt[:, q * 2592:(q + 1) * 2592],
                    xqb.ap()[:, q * 2592:(q + 1) * 2592])
            nc.gpsimd.dma_start(wct[:], wc.ap()[:])
            nc.gpsimd.dma_start(btile[:], bq.ap()[:])
            nc.gpsimd.dma_start(xct[:, 0:1296], xqc.ap()[:, 0:1296])
            for t0 in range(1, 4):
                nc.gpsimd.dma_start(xat[:, t0 * 5184:(t0 + 1) * 5184],
                                    xqa.ap()[:, t0 * 5184:(t0 + 1) * 5184])
                nc.gpsimd.dma_start(xbt[:, t0 * 5184:(t0 + 1) * 5184],
                                    xqb.ap()[:, t0 * 5184:(t0 + 1) * 5184])
                nc.gpsimd.dma_start(
                    xct[:, t0 * 1296:(t0 + 1) * 1296],
                    xqc.ap()[:, t0 * 1296:(t0 + 1) * 1296])

            xav = xat.rearrange("p (t d h w) -> p t d h w",
                                t=4, d=16, h=18, w=18)
            xbv = xbt.rearrange("p (t d h w) -> p t d h w",
                                t=4, d=16, h=18, w=18)
            xcv = xct.rearrange("p (t d h w) -> p t d h w",
                                t=4, d=4, h=18, w=18)

            for to in range(4):
                ps = [pp.tile([128, 512], f32, tag="ps",
                              name=f"ps_{to}_{dp}") for dp in range(8)]
                nch = [[0, 0] for _ in range(8)]
                tot = [[0, 0] for _ in range(8)]
                for pi in range(3):
                    for j in range(9):
                        c = (j + (1 if pi == 1 else 0)) % 2
                        for dp in range(8):
                            tot[dp][c] += 1

                def mm_ab(pi, j, dp):
                    wt, xv = ((wat, xav), (wbt, xbv))[pi]
                    kh, kw = KHW[j]
                    c = (j + pi) % 2
                    nch[dp][c] += 1
                    nc.tensor.matmul(
                        ps[dp][64 * c:64 * c + 64, :],
                        wt[:, j * 64:(j + 1) * 64],
                        xv[:, to, 2 * dp:2 * dp + 2,
                           kh:kh + 16, kw:kw + 16],
                        start=nch[dp][c] == 1,
                        stop=nch[dp][c] == tot[dp][c],
                        tile_position=(0, 64 * c))

                # (E,O)-paired issue so both column halves stream from the
                # first instruction; (A j8, B j0) bridges the pass change.
                PAIRS = [((0, 0), (0, 1)), ((0, 2), (0, 3)),
                         ((0, 4), (0, 5)), ((0, 6), (0, 7)),
                         ((0, 8), (1, 0)), ((1, 1), (1, 2)),
                         ((1, 3), (1, 4)), ((1, 5), (1, 6)),
                         ((1, 7), (1, 8))]
                DPO = (0, 1, 2, 3, 4, 5, 6, 7) if to == 0 else \
                    (0, 2, 4, 6, 1, 3, 5, 7)
                for (pa, ja), (pb, jb) in PAIRS:
                    for dp in DPO:
                        mm_ab(pa, ja, dp)
                        mm_ab(pb, jb, dp)
                # pass C: two waves of units spread over the 4 row groups
                for dp in (0, 2, 4, 6, 1, 3, 5, 7):
                    r = dp // 2
                    ld = 2 * (dp % 2)
                    for j, (kh, kw) in enumerate(KHW):
                        c = j % 2
                        nch[dp][c] += 1
                        nc.tensor.matmul(
                            ps[dp][64 * c:64 * c + 64, :],
                            wct[32 * r:32 * r + 32, j * 64:(j + 1) * 64],
                            xcv[32 * r:32 * r + 32, to, ld:ld + 2,
                                kh:kh + 16, kw:kw + 16],
                            start=nch[dp][c] == 1,
                            stop=nch[dp][c] == tot[dp][c],
                            tile_position=(32 * r, 64 * c))
                for dp in (0, 2, 4, 6, 1, 3, 5, 7):
                    oc = op_.tile([128, 512], f32, tag="oc",
                                  name=f"oc_{to}_{dp}")
                    oa = op_.tile([64, 512], f32, tag="oa",
                                  name=f"oa_{to}_{dp}")
                    # one full-bank read frees the PSUM bank
                    # immediately; bias rides along on the O-half
                    ob = op_.tile([64, 512], f32, tag="obs",
                                  name=f"obs_{to}_{dp}")
                    nc.scalar.activation(
                        oc[:], ps[dp][:, :],
                        mybir.ActivationFunctionType.Identity,
                        bias=btile[:, 0:1])
                    nc.scalar.dma_start(ob[:], oc[64:128, :])
                    nc.vector.tensor_tensor(oa[:], oc[0:64, :], ob[:],
                                            mybir.AluOpType.add)
                    off = to * 4096 + dp * 512
                    dq = nc.sync if dp % 4 < 2 else nc.gpsimd
                    dq.dma_start(out.ap()[:, off:off + 512], oa[:])
    nc.compile()
    _NC = nc
    return nc


def _prep_inputs(x, weight, bias):
    x = np.asarray(x, dtype=np.float32)
    weight = np.asarray(weight, dtype=np.float32)
    bias = np.asarray(bias, dtype=np.float32)

    def wpack(kt, kd):
        # [32ci, 9khw * 64co]
        return np.ascontiguousarray(
            weight[:, :, kt, kd].reshape(64, 32, 9).transpose(1, 2, 0)
        ).reshape(32, 576)

    wa = np.concatenate([wpack(kt, kd) for kt, kd in A_COMBOS], axis=0)
    wb = np.concatenate([wpack(kt, kd) for kt, kd in B_COMBOS], axis=0)
    wc = np.concatenate([wpack(1, 1)] * 4, axis=0)
    wa = wa.astype(ml_dtypes.bfloat16)
    wb = wb.astype(ml_dtypes.bfloat16)
    wc = wc.astype(ml_dtypes.bfloat16)
    bq = np.concatenate([np.zeros((64, 1), np.float32),
                     bias.reshape(64, 1)]).astype(np.float32)

    in_maps = []
    for b in range(2):
        xpad = np.pad(x[b], ((0, 0), (1, 1), (1, 1), (1, 1), (1, 1)))
        for tq in range(4):
            xt = xpad[:, 4 * tq:4 * tq + 6]  # [32, 6t, 18d, 18, 18]
            xa = np.empty((128, 20736), ml_dtypes.bfloat16)
            xb = np.empty((128, 20736), ml_dtypes.bfloat16)
            for g, (kt, kd) in enumerate(A_COMBOS):
                xa[32 * g:32 * g + 32] = \
                    xt[:, kt:kt + 4, kd:kd + 16].reshape(32, -1)
            for g, (kt, kd) in enumerate(B_COMBOS):
                xb[32 * g:32 * g + 32] = \
                    xt[:, kt:kt + 4, kd:kd + 16].reshape(32, -1)
            # cropped quadrant layout for pass C (kt=kd=1):
            # t planes 1..4, per-quarter padded-d planes 4r+1..4r+4
            xc = np.empty((128, 5184), ml_dtypes.bfloat16)
            for r in range(4):
                xc[32 * r:32 * r + 32] = \
                    xt[:, 1:5, 4 * r + 1:4 * r + 5].reshape(32, -1)
            in_maps.append({"xqa": xa, "xqb": xb, "xqc": xc,
                            "wa": wa, "wb": wb, "wc": wc, "biasq": bq})
    return in_maps


def run_spmd(x, weight, bias, trace=False, trace_cores=None, tmpdir=None):
    """Returns (output ndarray, BassKernelResults)."""
    from concourse.bass_utils import run_bass_kernel_spmd
    nc = _build()
    in_maps = _prep_inputs(x, weight, bias)
    res = run_bass_kernel_spmd(nc, in_maps, core_ids=list(range(N_CORES)),
                               trace=trace, trace_cores=trace_cores,
                               tmpdir=tmpdir)
    out = np.empty((2, 64, 16, 16, 16, 16), np.float32)
    for c in range(N_CORES):
        b, tq = c // 4, c % 4
        out[b, :, 4 * tq:4 * tq + 4] = \
            res.results[c]["out"].reshape(64, 4, 16, 16, 16)
    return out, res


def kernel(x, weight, bias):
    out, _ = run_spmd(x, weight, bias)
    return out


# revision 15
# speedup vs baseline: 1.2866x; 1.2866x over previous
"""Conv4d (B=2, Ci=32, Co=64, 16^4 spatial, k=3^4, stride 1, pad 1) on 8
Trainium2 NeuronCores.

Sharding: 8 cores = batch(2) x T-quarters(4). Each core computes
out[64co, 4t, 16d, 16h, 16w] for its (b, t-quarter).

The 81 taps are covered by three passes sized to keep the tensor engine
instruction count low (the ~31ns/inst issue rate binds at K=32):
  A: (kt,kd) in {(0,0),(0,1),(0,2),(1,0)} packed into K=128 (partition
     group g holds x shifted by combo g), M=64, one matmul per (kh,kw).
  B: (kt,kd) in {(1,2),(2,0),(2,1),(2,2)} likewise on a second layout.
  C: (kt,kd)=(1,1) as K=32 matmuls on a cropped quadrant layout
     (partition group r = D-quarter), 4 row groups concurrent, issued
     in two unit-waves so epilogues stagger.
Each pass splits its 9 (kh,kw) taps across the two 64-wide PE column
groups (tile_position col 0 / 64), accumulating even taps into PSUM
partitions 0-63 and odd taps into 64-127 of one [128,512] bank per
(to, dp) output unit; 8 units (one `to` batch) live at once = 8 banks.
Epilogue: ACT adds bias to the odd half, DVE adds the halves, DMA out.
"""
import sys

sys.path.insert(0, "/opt/trn_rl_repo")
import numpy as np
import ml_dtypes

N_CORES = 8
KHW = [(kh, kw) for kh in range(3) for kw in range(3)]
A_COMBOS = [(0, 0), (0, 1), (0, 2), (1, 0)]
B_COMBOS = [(1, 2), (2, 0), (2, 1), (2, 2)]

_NC = None


def _build():
    global _NC
    if _NC is not None:
        return _NC
    import concourse.bacc as bacc
    import concourse.tile as tile
    from concourse import mybir

    f32 = mybir.dt.float32
    bf16 = mybir.dt.bfloat16

    nc = bacc.Bacc("TRN2", debug=False, target_bir_lowering=False,
                   num_devices=N_CORES)
    xqa = nc.dram_tensor("xqa", [128, 20736], bf16, kind="ExternalInput")
    xqb = nc.dram_tensor("xqb", [128, 20736], bf16, kind="ExternalInput")
    xqc = nc.dram_tensor("xqc", [128, 5184], bf16, kind="ExternalInput")
    wa = nc.dram_tensor("wa", [128, 576], bf16, kind="ExternalInput")
    wb = nc.dram_tensor("wb", [128, 576], bf16, kind="ExternalInput")
    wc = nc.dram_tensor("wc", [128, 576], bf16, kind="ExternalInput")
    bq = nc.dram_tensor("biasq", [128, 1], f32, kind="ExternalInput")
    out = nc.dram_tensor("out", [64, 16384], f32, kind="ExternalOutput")

    with tile.TileContext(nc) as tc:
        with tc.tile_pool(name="xp", bufs=1) as xp, \
             tc.tile_pool(name="wp", bufs=1) as wp, \
             tc.tile_pool(name="op", bufs=8) as op_, \
             tc.tile_pool(name="pp", bufs=8, space="PSUM") as pp:
            wat = wp.tile([128, 576], bf16)
            wbt = wp.tile([128, 576], bf16)
            wct = wp.tile([128, 576], bf16)
            btile = wp.tile([128, 1], f32)
            xat = xp.tile([128, 20736], bf16)
            xbt = xp.tile([128, 20736], bf16)
            xct = xp.tile([128, 5184], bf16)

            # Issue order == arrival order (one FIFO input queue feeding
            # all 16 DMA engines). First matmuls need wa + A[t0=0,d 0..3]
            # only, so those go first, d-chunked, on the sync queue in
            # case it starts ahead of gpsimd.
            nc.sync.dma_start(wat[:], wa.ap()[:])
            for q in range(4):
                nc.sync.dma_start(xat[:, q * 1296:(q + 1) * 1296],
                                  xqa.ap()[:, q * 1296:(q + 1) * 1296])
            nc.gpsimd.dma_start(wbt[:], wb.ap()[:])
            for q in range(2):
                nc.gpsimd.dma_start(
                    xbt[:, q * 2592:(q + 1) * 2592],
                    xqb.ap()[:, q * 2592:(q + 1) * 2592])
            nc.gpsimd.dma_start(wct[:], wc.ap()[:])
            nc.gpsimd.dma_start(btile[:], bq.ap()[:])
            nc.gpsimd.dma_start(xct[:, 0:1296], xqc.ap()[:, 0:1296])
            for t0 in range(1, 4):
                nc.gpsimd.dma_start(xat[:, t0 * 5184:(t0 + 1) * 5184],
                                    xqa.ap()[:, t0 * 5184:(t0 + 1) * 5184])
                nc.gpsimd.dma_start(xbt[:, t0 * 5184:(t0 + 1) * 5184],
                                    xqb.ap()[:, t0 * 5184:(t0 + 1) * 5184])
                nc.gpsimd.dma_start(
                    xct[:, t0 * 1296:(t0 + 1) * 1296],
                    xqc.ap()[:, t0 * 1296:(t0 + 1) * 1296])

            xav = xat.rearrange("p (t d h w) -> p t d h w",
                                t=4, d=16, h=18, w=18)
            xbv = xbt.rearrange("p (t d h w) -> p t d h w",
                                t=4, d=16, h=18, w=18)
            xcv = xct.rearrange("p (t d h w) -> p t d h w",
                                t=4, d=4, h=18, w=18)

            # 8 batches of 4 same-parity units; consecutive batches use
            # disjoint PSUM banks so the tensor queue never waits on an
            # epilogue. Same-parity units put pass C on all 4 row groups.
            for bi in range(8):
                to, par = bi // 2, bi % 2
                units = [par, par + 2, par + 4, par + 6]
                ps = {dp: pp.tile([128, 512], f32, tag="ps",
                                  name=f"ps_{to}_{dp}") for dp in units}
                nch = {dp: [0, 0] for dp in units}
                tot = {dp: [14, 13] for dp in units}

                def mm_ab(pi, j, dp):
                    wt, xv = ((wat, xav), (wbt, xbv))[pi]
                    kh, kw = KHW[j]
                    c = (j + pi) % 2
                    nch[dp][c] += 1
                    nc.tensor.matmul(
                        ps[dp][64 * c:64 * c + 64, :],
                        wt[:, j * 64:(j + 1) * 64],
                        xv[:, to, 2 * dp:2 * dp + 2,
                           kh:kh + 16, kw:kw + 16],
                        start=nch[dp][c] == 1,
                        stop=nch[dp][c] == tot[dp][c],
                        tile_position=(0, 64 * c))

                # (E,O)-paired issue so both column halves stream from the
                # first instruction; (A j8, B j0) bridges the pass change.
                PAIRS = [((0, 0), (0, 1)), ((0, 2), (0, 3)),
                         ((0, 4), (0, 5)), ((0, 6), (0, 7)),
                         ((0, 8), (1, 0)), ((1, 1), (1, 2)),
                         ((1, 3), (1, 4)), ((1, 5), (1, 6)),
                         ((1, 7), (1, 8))]
                for (pa, ja), (pb, jb) in PAIRS:
                    for dp in units:
                        mm_ab(pa, ja, dp)
                        mm_ab(pb, jb, dp)
                # pass C: each unit on its own row group, all concurrent
                for dp in units:
                    r = dp // 2
                    ld = 2 * (dp % 2)
                    for j, (kh, kw) in enumerate(KHW):
                        c = j % 2
                        nch[dp][c] += 1
                        nc.tensor.matmul(
                            ps[dp][64 * c:64 * c + 64, :],
                            wct[32 * r:32 * r + 32, j * 64:(j + 1) * 64],
                            xcv[32 * r:32 * r + 32, to, ld:ld + 2,
                                kh:kh + 16, kw:kw + 16],
                            start=nch[dp][c] == 1,
                            stop=nch[dp][c] == tot[dp][c],
                            tile_position=(32 * r, 64 * c))
                for dp in units:
                    ob = op_.tile([64, 512], f32, tag="ob",
                                  name=f"ob_{to}_{dp}")
                    oa = op_.tile([64, 512], f32, tag="oa",
                                  name=f"oa_{to}_{dp}")
                    nc.scalar.activation(
                        ob[:], ps[dp][64:128, :],
                        mybir.ActivationFunctionType.Identity,
                        bias=btile[64:128, 0:1])
                    nc.vector.tensor_tensor(oa[:], ps[dp][0:64, :],
                                            ob[:], mybir.AluOpType.add)
                    off = to * 4096 + dp * 512
                    dq = nc.sync if dp < 4 else nc.gpsimd
                    dq.dma_start(out.ap()[:, off:off + 512], oa[:])
    nc.compile()
    _NC = nc
    return nc


def _prep_inputs(x, weight, bias):
    x = np.asarray(x, dtype=np.float32)
    weight = np.asarray(weight, dtype=np.float32)
    bias = np.asarray(bias, dtype=np.float32)

    def wpack(kt, kd):
        # [32ci, 9khw * 64co]
        return np.ascontiguousarray(
            weight[:, :, kt, kd].reshape(64, 32, 9).transpose(1, 2, 0)
        ).reshape(32, 576)

    wa = np.concatenate([wpack(kt, kd) for kt, kd in A_COMBOS], axis=0)
    wb = np.concatenate([wpack(kt, kd) for kt, kd in B_COMBOS], axis=0)
    wc = np.concatenate([wpack(1, 1)] * 4, axis=0)
    wa = wa.astype(ml_dtypes.bfloat16)
    wb = wb.astype(ml_dtypes.bfloat16)
    wc = wc.astype(ml_dtypes.bfloat16)
    bq = np.concatenate([np.zeros((64, 1), np.float32),
                     bias.reshape(64, 1)]).astype(np.float32)

    in_maps = []
    for b in range(2):
        xpad = np.pad(x[b], ((0, 0), (1, 1), (1, 1), (1, 1), (1, 1)))
        for tq in range(4):
            xt = xpad[:, 4 * tq:4 * tq + 6]  # [32, 6t, 18d, 18, 18]
            xa = np.empty((128, 20736), ml_dtypes.bfloat16)
            xb = np.empty((128, 20736), ml_dtypes.bfloat16)
            for g, (kt, kd) in enumerate(A_COMBOS):
                xa[32 * g:32 * g + 32] = \
                    xt[:, kt:kt + 4, kd:kd + 16].reshape(32, -1)
            for g, (kt, kd) in enumerate(B_COMBOS):
                xb[32 * g:32 * g + 32] = \
                    xt[:, kt:kt + 4, kd:kd + 16].reshape(32, -1)
            # cropped quadrant layout for pass C (kt=kd=1):
            # t planes 1..4, per-quarter padded-d planes 4r+1..4r+4
            xc = np.empty((128, 5184), ml_dtypes.bfloat16)
            for r in range(4):
                xc[32 * r:32 * r + 32] = \
                    xt[:, 1:5, 4 * r + 1:4 * r + 5].reshape(32, -1)
            in_maps.append({"xqa": xa, "xqb": xb, "xqc": xc,
                            "wa": wa, "wb": wb, "wc": wc, "biasq": bq})
    return in_maps


def run_spmd(x, weight, bias, trace=False, trace_cores=None, tmpdir=None):
    """Returns (output ndarray, BassKernelResults)."""
    from concourse.bass_utils import run_bass_kernel_spmd
    nc = _build()
    in_maps = _prep_inputs(x, weight, bias)
    res = run_bass_kernel_spmd(nc, in_maps, core_ids=list(range(N_CORES)),
                               trace=trace, trace_cores=trace_cores,
                               tmpdir=tmpdir)
    out = np.empty((2, 64, 16, 16, 16, 16), np.float32)
    for c in range(N_CORES):
        b, tq = c // 4, c % 4
        out[b, :, 4 * tq:4 * tq + 4] = \
            res.results[c]["out"].reshape(64, 4, 16, 16, 16)
    return out, res


def kernel(x, weight, bias):
    out, _ = run_spmd(x, weight, bias)
    return out
